# revision 1
# baseline (speedup 1.0000x reference)
"""4-bit column-block-quantized linear (ColBlockQuantizedLinear) on 8 Trainium2 NeuronCores.

Reference computation:
    w[n, k] = (nibble(quant_weight)[n, k] - zeros[n]) * scales[n]     n<11008, k<4096
    out[b, s, n] = sum_k inp[b, s, k] * w[n, k]                        inp: [4, 2048, 4096] f32

Strategy (column-parallel, per sharding hint):
  - Shard out_features N=11008 = 8*1376 across 8 cores; replicate inp.
  - Host-side layout prep only: transpose/permute inp to k-major bf16, cast packed
    weights int32->uint8 and transpose to [k/2, n] per core, row-sums of inp,
    broadcast scale rows.
  - On-chip per core: unpack nibbles into resident SBUF weight tiles holding the
    RAW 4-bit values (exact in bf16), then a dense bf16 matmul accumulating
    psum[m, n] += xT[k, m].T @ Q[k, n] over 32 k-tiles. Dequant is folded into
    the f32 PSUM eviction:  out = psum * s[n] + (-s[n]*z[n]) * rowsum[m],
    which is exact up to the bf16 rounding of the activations.
  - A short burst of dummy matmuls warms the PE (HAM un-throttle) while the
    first activation tiles and weights stream in.
  - Host concatenates per-core outputs along N.
"""

import sys

for _p in ("/opt/trn_rl_repo", "/opt/pypackages"):
    if _p not in sys.path:
        sys.path.append(_p)

import numpy as np
import ml_dtypes

import concourse.bass as bass
import concourse.mybir as mybir
import concourse.tile as tile
from concourse import bacc

# Problem constants (hardcoded per harness contract)
B, S, K = 4, 2048, 4096
M = B * S                  # 8192 tokens
N = 11008                  # out features
NCORES = 8
NPC = N // NCORES          # per-core out features (1376)
KP = K // 2                # packed k rows (2048)
P = 128


def _nchunks(npc, wide=False):
    step = 1024 if wide else 512
    return [(i, min(step, npc - i)) for i in range(0, npc, step)]


def build_nc(m=M, kp=KP, npc=NPC, mg=512, warmup=120, wide=False):
    """Build the per-core Bass program. m tokens, kp packed-k rows, npc out cols,
    mg tokens per m-group (DMA granule)."""
    ktp = kp // P              # packed k tiles (16)
    kt_n = 2 * ktp             # unpacked k tiles (32)
    ngroups = m // mg
    mbs = mg // P              # m-blocks per group
    chunks = _nchunks(npc, wide)

    nc = bacc.Bacc("TRN2", target_bir_lowering=False, debug=False)
    xt_d = nc.dram_tensor("xt", [kt_n, P, m], mybir.dt.bfloat16, kind="ExternalInput")
    qwt_d = nc.dram_tensor("qwt", [ktp, P, npc], mybir.dt.uint8, kind="ExternalInput")
    s_d = nc.dram_tensor("s32", [P, npc], mybir.dt.float32, kind="ExternalInput")
    nb_d = nc.dram_tensor("nb32", [P, npc], mybir.dt.float32, kind="ExternalInput")
    rs_d = nc.dram_tensor("rs", [P, m // P], mybir.dt.float32, kind="ExternalInput")
    out_d = nc.dram_tensor("out", [m, npc], mybir.dt.float32, kind="ExternalOutput")

    with tile.TileContext(nc) as tc:
        with (
            tc.tile_pool(name="const", bufs=1) as const_pool,
            tc.tile_pool(name="stage", bufs=2) as stage_pool,
            tc.tile_pool(name="w", bufs=1) as w_pool,
            tc.tile_pool(name="x", bufs=2) as x_pool,
            tc.tile_pool(name="o", bufs=2) as o_pool,
            tc.tile_pool(name="ps", bufs=2, space="PSUM") as ps_pool,
            tc.tile_pool(name="wps", bufs=1, space="PSUM") as warm_ps_pool,
        ):
            s32t = const_pool.tile([P, npc], mybir.dt.float32, tag="s32t")
            nb32t = const_pool.tile([P, npc], mybir.dt.float32, tag="nb32t")
            rs_t = const_pool.tile([P, m // P], mybir.dt.float32, tag="rs_t")
            # PE warmup: flip the HAM clock gate to 8/8 while DMAs/unpack run.
            if warmup:
                wsrc = const_pool.tile([P, 256], mybir.dt.bfloat16, tag="wsrc")
                nc.vector.memset(wsrc[:], 0.0)
                wp = warm_ps_pool.tile([P, 256], mybir.dt.float32, tag="wp")
                for _ in range(warmup):
                    nc.tensor.matmul(wp[:], wsrc[:, :P], wsrc[:], start=True, stop=True)

            xg0 = x_pool.tile([P, kt_n, mg], mybir.dt.bfloat16, tag="xg")

            # Unpack raw nibbles into resident SBUF tiles (values 0..15, exact
            # in bf16). W[kt] for kt in [0, ktp) = low nibbles (even k),
            # [ktp, 2ktp) = high nibbles (odd k).
            w_tiles = [
                w_pool.tile([P, npc], mybir.dt.bfloat16, name=f"W{kt}", tag=f"W{kt}")
                for kt in range(kt_n)
            ]
            for kt in range(ktp):
                q = stage_pool.tile([P, npc], mybir.dt.uint8, tag="q")
                nc.sync.dma_start(q[:], qwt_d[kt])
                # bitwise must be dtype-preserving; do it 2 packed bytes per
                # lane-op via a u16 view (masks are byte-symmetric), then
                # cast u8->bf16 on ScalarE
                lo8 = stage_pool.tile([P, npc], mybir.dt.uint8, tag="lo8")
                hi8 = stage_pool.tile([P, npc], mybir.dt.uint8, tag="hi8")
                nc.vector.tensor_scalar(
                    lo8[:].bitcast(mybir.dt.uint16), q[:].bitcast(mybir.dt.uint16),
                    0x0F0F, None, op0=mybir.AluOpType.bitwise_and,
                )
                nc.vector.tensor_scalar(
                    hi8[:].bitcast(mybir.dt.uint16), q[:].bitcast(mybir.dt.uint16),
                    4, 0x0F0F,
                    op0=mybir.AluOpType.logical_shift_right,
                    op1=mybir.AluOpType.bitwise_and,
                )
                nc.scalar.copy(w_tiles[kt][:], lo8[:])
                nc.scalar.copy(w_tiles[ktp + kt][:], hi8[:])
                # two g0 activation DMAs per weight tile: interleaved so the
                # first m-group lands early without starving the weight queue
                nc.sync.dma_start(xg0[:, 2 * kt, :], xt_d[2 * kt, :, 0:mg])
                nc.sync.dma_start(xg0[:, 2 * kt + 1, :], xt_d[2 * kt + 1, :, 0:mg])

            # scale rows are first needed at the first eviction (~50us in):
            # keep their 1.4MB out of the pre-mb0 DMA critical path
            nc.sync.dma_start(s32t[:], s_d[:])
            nc.sync.dma_start(nb32t[:], nb_d[:])
            nc.sync.dma_start(rs_t[:], rs_d[:])

            # Main matmul loop: m-groups of `mg` tokens, 128-token m-blocks.
            # k-tiles consumed in unpack-completion order (low_t, high_t).
            kt_order = [t + h * ktp for t in range(ktp) for h in (0, 1)]
            for g in range(ngroups):
                if g == 0:
                    xg = xg0
                else:
                    xg = x_pool.tile([P, kt_n, mg], mybir.dt.bfloat16, tag="xg")
                    for kt in range(kt_n):
                        nc.sync.dma_start(
                            xg[:, kt, :], xt_d[kt, :, g * mg:(g + 1) * mg]
                        )
                for mb in range(mbs):
                    mbi = g * mbs + mb
                    ps = ps_pool.tile([P, npc], mybir.dt.float32, tag="ps")
                    for i, kt in enumerate(kt_order):
                        lhsT = xg[:, kt, mb * P:(mb + 1) * P]
                        for (n0, nw) in chunks:
                            nc.tensor.matmul(
                                ps[:, n0:n0 + nw], lhsT, w_tiles[kt][:, n0:n0 + nw],
                                start=(i == 0), stop=(i == kt_n - 1),
                            )
                    # eviction (ACT, keeps PSUM-read pattern cheap for PE),
                    # then dequant on SBUF:  out = psum * s + (-s*z) * rowsum[m]
                    ot = o_pool.tile([P, npc], mybir.dt.float32, tag="ot")
                    nc.scalar.copy(ot[:], ps[:])
                    nc.vector.tensor_tensor(
                        ot[:], ot[:], s32t[:], op=mybir.AluOpType.mult
                    )
                    nc.vector.scalar_tensor_tensor(
                        ot[:], nb32t[:], rs_t[:, mbi:mbi + 1], ot[:],
                        op0=mybir.AluOpType.mult, op1=mybir.AluOpType.add,
                    )
                    m0 = g * mg + mb * P
                    nc.sync.dma_start(out_d[m0:m0 + P, :], ot[:])

    nc.compile()
    return nc


def prep_inputs(inp, quant_weight, scales, zeros, ncores=NCORES, npc=NPC):
    """Host-side sharding/layout: returns in_maps list for run_bass_kernel_spmd."""
    m = inp.shape[0] * inp.shape[1]
    k = inp.shape[2]
    kp = k // 2
    ktp = kp // P

    x = np.asarray(inp, dtype=np.float32).reshape(m, k)
    x3 = x.reshape(m, kp, 2)
    # xt rows: kt in [0, ktp) -> even k (low nibble), [ktp, 2ktp) -> odd k (high)
    xt_even = np.ascontiguousarray(x3[:, :, 0].T).astype(ml_dtypes.bfloat16)
    xt_odd = np.ascontiguousarray(x3[:, :, 1].T).astype(ml_dtypes.bfloat16)
    xt = np.concatenate(
        [xt_even.reshape(ktp, P, m), xt_odd.reshape(ktp, P, m)], axis=0
    )  # [2*ktp, P, m] bf16

    # rowsum of the exact activations, for the zero-point correction term
    rs = x.sum(axis=1, dtype=np.float64).astype(np.float32)  # [m]
    rs_host = np.ascontiguousarray(rs.reshape(m // P, P).T)  # [P, m//P]

    n = quant_weight.shape[0]
    assert n == ncores * npc, (n, ncores, npc)
    qw8 = np.asarray(quant_weight).astype(np.uint8)
    s_all = np.asarray(scales, dtype=np.float32).reshape(-1)
    z_all = np.asarray(zeros, dtype=np.float32).reshape(-1)
    nb_all = -(s_all * z_all)

    in_maps = []
    for c in range(ncores):
        sl = slice(c * npc, (c + 1) * npc)
        qwt_c = np.ascontiguousarray(qw8[sl].T).reshape(ktp, P, npc)
        s_c = np.ascontiguousarray(np.broadcast_to(s_all[sl], (P, npc)))
        nb_c = np.ascontiguousarray(np.broadcast_to(nb_all[sl], (P, npc)))
        in_maps.append(
            {"xt": xt, "qwt": qwt_c, "s32": s_c, "nb32": nb_c, "rs": rs_host}
        )
    return in_maps


_NC_CACHE = {}


def _get_nc():
    if "nc" not in _NC_CACHE:
        _NC_CACHE["nc"] = build_nc()
    return _NC_CACHE["nc"]


def kernel(inp, quant_weight, scales, zeros):
    from concourse.bass_utils import run_bass_kernel_spmd

    nc = _get_nc()
    in_maps = prep_inputs(inp, quant_weight, scales, zeros)
    res = run_bass_kernel_spmd(nc, in_maps, list(range(NCORES)))
    out = np.concatenate([res.results[c]["out"] for c in range(NCORES)], axis=1)
    return np.ascontiguousarray(out).reshape(B, S, N)



# revision 3
# speedup vs baseline: 1.7402x; 1.7402x over previous
"""4-bit column-block-quantized linear (ColBlockQuantizedLinear) on 8 Trainium2 NeuronCores.

Reference computation:
    w[n, k] = (nibble(quant_weight)[n, k] - zeros[n]) * scales[n]     n<11008, k<4096
    out[b, s, n] = sum_k inp[b, s, k] * w[n, k]                        inp: [4, 2048, 4096] f32

Strategy (column-parallel, per sharding hint), fp8 DoubleRow edition:
  - Shard out_features N=11008 = 8*1376 across 8 cores; replicate inp.
  - The nibble weights q in {0..15} are EXACT in fp8-e4m3.  Activations are
    quantized host-side to x8 = e4m3(32*x).  The PE then runs in DoubleRow
    perf mode (two 128-row k-tiles per pass, 2x bf16 throughput):
        psum[m, n] += sum_i x8[kpair, :, i, m].T @ q[kpair, :, i, n]
    over 16 k-pairs (K = 4096 = 16 pairs * 256).
  - Error control: the activation-quantization residual xlo = x - x8/32 is
    handled two ways:  (a) for the first L k-pairs an exact second DoubleRow
    pass with xlo8 = e4m3(512*xlo) against q/16 (also exact in fp8)
    accumulates into the SAME psum group;  (b) the remaining residual is
    corrected at rank 1 with exact per-token residual rowsums against the
    exact per-channel mean nibble qbar_n.
  - Eviction fuses the dequant:  out = psum*(s/32) + (s*qbar)*rl[m] + (-s*z)*r[m]
    with r = exact rowsums of x, rl = rowsums of the remaining residual.
  - A short burst of dummy matmuls warms the PE (HAM un-throttle) while the
    first activation tiles and weights stream in.
  - Host concatenates per-core outputs along N.
"""

import sys

for _p in ("/opt/trn_rl_repo", "/opt/pypackages"):
    if _p not in sys.path:
        sys.path.append(_p)

import numpy as np
import ml_dtypes

import concourse.bass as bass
import concourse.mybir as mybir
import concourse.tile as tile
from concourse import bacc

# Problem constants (hardcoded per harness contract)
B, S, K = 4, 2048, 4096
M = B * S                  # 8192 tokens
N = 11008                  # out features
NCORES = 8
NPC = N // NCORES          # per-core out features (1376)
P = 128
NPAIR = K // (2 * P)       # 16 DoubleRow k-pairs
LO_PAIRS = 2               # k-pairs that get an exact fp8 residual pass
C_HI = 32.0                # x8 = e4m3(C_HI * x)
C_LO = 512.0               # xlo8 = e4m3(C_LO * (x - x8/C_HI)); C_LO/C_HI = 16


def _nchunks(npc, step=512):
    return [(i, min(step, npc - i)) for i in range(0, npc, step)]


def build_nc(m=M, npc=NPC, lo_pairs=LO_PAIRS, mg=1024, warmup=120):
    """Build the per-core Bass program. m tokens, npc out cols, mg tokens per
    m-group (DMA granule)."""
    ngroups = m // mg
    mbs = mg // P              # m-blocks per group
    chunks = _nchunks(npc)
    npass = NPAIR + lo_pairs
    DR = mybir.MatmulPerfMode.DoubleRow

    nc = bacc.Bacc("TRN2", target_bir_lowering=False, debug=False)
    xt_d = nc.dram_tensor("xt", [NPAIR, P, 2, m], mybir.dt.float8e4, kind="ExternalInput")
    qt_d = nc.dram_tensor("qt", [NPAIR, P, 2, npc], mybir.dt.float8e4, kind="ExternalInput")
    if lo_pairs:
        xl_d = nc.dram_tensor("xl", [lo_pairs, P, 2, m], mybir.dt.float8e4, kind="ExternalInput")
        ql_d = nc.dram_tensor("ql", [lo_pairs, P, 2, npc], mybir.dt.float8e4, kind="ExternalInput")
    a_d = nc.dram_tensor("a32", [P, npc], mybir.dt.float32, kind="ExternalInput")
    b_d = nc.dram_tensor("b32", [P, npc], mybir.dt.float32, kind="ExternalInput")
    c_d = nc.dram_tensor("c32", [P, npc], mybir.dt.float32, kind="ExternalInput")
    rs_d = nc.dram_tensor("rs", [P, m // P], mybir.dt.float32, kind="ExternalInput")
    rl_d = nc.dram_tensor("rl", [P, m // P], mybir.dt.float32, kind="ExternalInput")
    out_d = nc.dram_tensor("out", [m, npc], mybir.dt.float32, kind="ExternalOutput")

    with tile.TileContext(nc) as tc:
        with (
            tc.tile_pool(name="const", bufs=1) as const_pool,
            tc.tile_pool(name="w", bufs=1) as w_pool,
            tc.tile_pool(name="x", bufs=2) as x_pool,
            tc.tile_pool(name="o", bufs=2) as o_pool,
            tc.tile_pool(name="ps", bufs=2, space="PSUM") as ps_pool,
            tc.tile_pool(name="wps", bufs=1, space="PSUM") as warm_ps_pool,
        ):
            a32t = const_pool.tile([P, npc], mybir.dt.float32, tag="a32t")
            b32t = const_pool.tile([P, npc], mybir.dt.float32, tag="b32t")
            c32t = const_pool.tile([P, npc], mybir.dt.float32, tag="c32t")
            rs_t = const_pool.tile([P, m // P], mybir.dt.float32, tag="rs_t")
            rl_t = const_pool.tile([P, m // P], mybir.dt.float32, tag="rl_t")
            # PE warmup: flip the HAM clock gate to 8/8 while DMAs stream in.
            if warmup:
                wsrc = const_pool.tile([P, 256], mybir.dt.bfloat16, tag="wsrc")
                nc.vector.memset(wsrc[:], 0.0)
                wp = warm_ps_pool.tile([P, 256], mybir.dt.float32, tag="wp")
                for _ in range(warmup):
                    nc.tensor.matmul(wp[:], wsrc[:, :P], wsrc[:], start=True, stop=True)

            # Resident weights: 16 hi pairs + lo_pairs residual pairs, fp8.
            w_tiles = [
                w_pool.tile([P, 2, npc], mybir.dt.float8e4, name=f"W{pr}", tag=f"W{pr}")
                for pr in range(npass)
            ]
            # group-0 activations land interleaved with the weights so the
            # first passes can start while the rest stream in
            xg0 = x_pool.tile([P, npass, 2, mg], mybir.dt.float8e4, tag="xg")
            for pr in range(NPAIR):
                nc.sync.dma_start(w_tiles[pr][:], qt_d[pr])
                nc.sync.dma_start(xg0[:, pr], xt_d[pr, :, :, 0:mg])
            for j in range(lo_pairs):
                nc.sync.dma_start(w_tiles[NPAIR + j][:], ql_d[j])
                nc.sync.dma_start(xg0[:, NPAIR + j], xl_d[j, :, :, 0:mg])

            # coefficient rows are first needed at the first eviction: keep
            # their 2.1MB out of the pre-mb0 DMA critical path
            nc.sync.dma_start(a32t[:], a_d[:])
            nc.sync.dma_start(b32t[:], b_d[:])
            nc.sync.dma_start(c32t[:], c_d[:])
            nc.sync.dma_start(rs_t[:], rs_d[:])
            nc.sync.dma_start(rl_t[:], rl_d[:])

            # Main matmul loop: m-groups of `mg` tokens, 128-token m-blocks.
            for g in range(ngroups):
                if g == 0:
                    xg = xg0
                else:
                    xg = x_pool.tile([P, npass, 2, mg], mybir.dt.float8e4, tag="xg")
                    for pr in range(NPAIR):
                        nc.sync.dma_start(
                            xg[:, pr], xt_d[pr, :, :, g * mg:(g + 1) * mg]
                        )
                    for j in range(lo_pairs):
                        nc.sync.dma_start(
                            xg[:, NPAIR + j], xl_d[j, :, :, g * mg:(g + 1) * mg]
                        )
                for mb in range(mbs):
                    mbi = g * mbs + mb
                    ps = ps_pool.tile([P, npc], mybir.dt.float32, tag="ps")
                    for i in range(npass):
                        lhsT = xg[:, i, :, mb * P:(mb + 1) * P]
                        for (n0, nw) in chunks:
                            nc.tensor.matmul(
                                ps[:, n0:n0 + nw], lhsT,
                                w_tiles[i][:, :, n0:n0 + nw],
                                start=(i == 0), stop=(i == npass - 1),
                                perf_mode=DR,
                            )
                    # eviction (ACT), then fused dequant on DVE:
                    #   out = psum*(s/32) + (s*qbar)*rl[m] + (-s*z)*r[m]
                    ot = o_pool.tile([P, npc], mybir.dt.float32, tag="ot")
                    nc.scalar.copy(ot[:], ps[:])
                    nc.vector.tensor_tensor(
                        ot[:], ot[:], a32t[:], op=mybir.AluOpType.mult
                    )
                    nc.vector.scalar_tensor_tensor(
                        ot[:], b32t[:], rl_t[:, mbi:mbi + 1], ot[:],
                        op0=mybir.AluOpType.mult, op1=mybir.AluOpType.add,
                    )
                    nc.vector.scalar_tensor_tensor(
                        ot[:], c32t[:], rs_t[:, mbi:mbi + 1], ot[:],
                        op0=mybir.AluOpType.mult, op1=mybir.AluOpType.add,
                    )
                    m0 = g * mg + mb * P
                    nc.sync.dma_start(out_d[m0:m0 + P, :], ot[:])

    nc.compile()
    return nc


def _to_pairs(arr_km, npair):
    """[M-ish rows, K' cols] -> [npair, 128, 2, rows] with
    k' = 256*pair + 128*slot + partition_row."""
    rows = arr_km.shape[0]
    t = np.ascontiguousarray(
        arr_km.T.reshape(npair, 2, P, rows).transpose(0, 2, 1, 3)
    )
    return t


def prep_inputs(inp, quant_weight, scales, zeros, ncores=NCORES, npc=NPC,
                lo_pairs=LO_PAIRS):
    """Host-side sharding/layout: returns in_maps list for run_bass_kernel_spmd."""
    m = inp.shape[0] * inp.shape[1]
    k = inp.shape[2]
    kp = k // 2
    f8 = ml_dtypes.float8_e4m3

    x = np.asarray(inp, dtype=np.float32).reshape(m, k)
    # k' permutation: k' in [0, 2048) -> even k (low nibble), [2048, 4096) -> odd
    xp = np.concatenate([x[:, 0::2], x[:, 1::2]], axis=1)  # [m, k']
    x8 = (xp * np.float32(C_HI)).astype(f8)
    xlo = xp - x8.astype(np.float32) / np.float32(C_HI)
    xt = _to_pairs(x8, NPAIR)  # [16, P, 2, m] fp8

    klo = 2 * P * lo_pairs
    if lo_pairs:
        xl8 = (xlo[:, :klo] * np.float32(C_LO)).astype(f8)
        xlt = _to_pairs(xl8, lo_pairs)  # [lo_pairs, P, 2, m] fp8
        xlo[:, :klo] -= xl8.astype(np.float32) / np.float32(C_LO)

    r = x.sum(axis=1, dtype=np.float64).astype(np.float32)    # [m] exact rowsums
    rl = xlo.sum(axis=1, dtype=np.float64).astype(np.float32)  # residual rowsums
    rs_host = np.ascontiguousarray(r.reshape(m // P, P).T)    # [P, m//P]
    rl_host = np.ascontiguousarray(rl.reshape(m // P, P).T)

    n = quant_weight.shape[0]
    assert n == ncores * npc, (n, ncores, npc)
    qb = np.asarray(quant_weight).astype(np.uint8)
    low = (qb & 15).astype(np.float32)
    high = ((qb >> 4) & 15).astype(np.float32)
    qp = np.concatenate([low, high], axis=1)  # [n, k'] nibble values
    qbar = qp.mean(axis=1, dtype=np.float64).astype(np.float32)  # [n] exact

    s_all = np.asarray(scales, dtype=np.float32).reshape(-1)
    z_all = np.asarray(zeros, dtype=np.float32).reshape(-1)
    a_all = s_all / np.float32(C_HI)
    b_all = s_all * qbar
    c_all = -(s_all * z_all)

    in_maps = []
    for c in range(ncores):
        sl = slice(c * npc, (c + 1) * npc)
        qpc = qp[sl]
        qt_c = _to_pairs(qpc.astype(f8), NPAIR)  # [16, P, 2, npc] fp8, exact
        im = {
            "xt": xt, "qt": qt_c,
            "a32": np.ascontiguousarray(np.broadcast_to(a_all[sl], (P, npc))),
            "b32": np.ascontiguousarray(np.broadcast_to(b_all[sl], (P, npc))),
            "c32": np.ascontiguousarray(np.broadcast_to(c_all[sl], (P, npc))),
            "rs": rs_host, "rl": rl_host,
        }
        if lo_pairs:
            # residual pass moving operand: q/16, exact in e4m3
            im["xl"] = xlt
            im["ql"] = _to_pairs(
                (qpc[:, :klo] / np.float32(16.0)).astype(f8), lo_pairs
            )
        in_maps.append(im)
    return in_maps


_NC_CACHE = {}


def _get_nc():
    if "nc" not in _NC_CACHE:
        _NC_CACHE["nc"] = build_nc()
    return _NC_CACHE["nc"]


def kernel(inp, quant_weight, scales, zeros):
    from concourse.bass_utils import run_bass_kernel_spmd

    nc = _get_nc()
    in_maps = prep_inputs(inp, quant_weight, scales, zeros)
    res = run_bass_kernel_spmd(nc, in_maps, list(range(NCORES)))
    out = np.concatenate([res.results[c]["out"] for c in range(NCORES)], axis=1)
    return np.ascontiguousarray(out).reshape(B, S, N)


# revision 8
# speedup vs baseline: 1.8511x; 1.0637x over previous
"""4-bit column-block-quantized linear (ColBlockQuantizedLinear) on 8 Trainium2 NeuronCores.

Reference computation:
    w[n, k] = (nibble(quant_weight)[n, k] - zeros[n]) * scales[n]     n<11008, k<4096
    out[b, s, n] = sum_k inp[b, s, k] * w[n, k]                        inp: [4, 2048, 4096] f32

Strategy (column-parallel, per sharding hint), fp8 DoubleRow edition:
  - Shard out_features N=11008 = 8*1376 across 8 cores; replicate inp.
  - The nibble weights q in {0..15} are EXACT in fp8-e4m3.  Activations are
    quantized host-side to x8 = e4m3(32*x).  The PE then runs in DoubleRow
    perf mode (two 128-row k-tiles per pass, 2x bf16 throughput):
        psum[m, n] += sum_i x8[kpair, :, i, m].T @ q[kpair, :, i, n]
    over 16 k-pairs (K = 4096 = 16 pairs * 256).
  - Error control: the activation-quantization residual xlo = x - x8/32 is
    handled two ways:  (a) for the first L k-pairs an exact second DoubleRow
    pass with xlo8 = e4m3(512*xlo) against q/16 (also exact in fp8)
    accumulates into the SAME psum group;  (b) the remaining residual is
    corrected at rank 1 with exact per-token residual rowsums against the
    exact per-channel mean nibble qbar_n.
  - Eviction fuses the dequant:  out = psum*(s/32) + (s*qbar)*rl[m] + (-s*z)*r[m]
    with r = exact rowsums of x, rl = rowsums of the remaining residual.
  - A short burst of dummy matmuls warms the PE (HAM un-throttle) while the
    first activation tiles and weights stream in.
  - Host concatenates per-core outputs along N.
"""

import sys

for _p in ("/opt/trn_rl_repo", "/opt/pypackages"):
    if _p not in sys.path:
        sys.path.append(_p)

import numpy as np
import ml_dtypes

import concourse.bass as bass
import concourse.mybir as mybir
import concourse.tile as tile
from concourse import bacc

# Problem constants (hardcoded per harness contract)
B, S, K = 4, 2048, 4096
M = B * S                  # 8192 tokens
N = 11008                  # out features
NCORES = 8
NPC = N // NCORES          # per-core out features (1376)
P = 128
NPAIR = K // (2 * P)       # 16 DoubleRow k-pairs
LO_PAIRS = 1               # k-pairs that get an exact fp8 residual pass
C_HI = 32.0                # x8 = e4m3(C_HI * x)
C_LO = 512.0               # xlo8 = e4m3(C_LO * (x - x8/C_HI)); C_LO/C_HI = 16


def _nchunks(npc, step=512):
    return [(i, min(step, npc - i)) for i in range(0, npc, step)]


def build_nc(m=M, npc=NPC, lo_pairs=LO_PAIRS, mg=1024, warmup=24):
    """Build the per-core Bass program. m tokens, npc out cols, mg tokens per
    m-group (DMA granule)."""
    ngroups = m // mg
    mbs = mg // P              # m-blocks per group
    chunks = _nchunks(npc)
    npass = NPAIR + lo_pairs
    DR = mybir.MatmulPerfMode.DoubleRow

    nc = bacc.Bacc("TRN2", target_bir_lowering=False, debug=False)
    xt_d = nc.dram_tensor("xt", [NPAIR, P, 2, m], mybir.dt.float8e4, kind="ExternalInput")
    qt_d = nc.dram_tensor("qt", [NPAIR, P, 2, npc], mybir.dt.float8e4, kind="ExternalInput")
    if lo_pairs:
        xl_d = nc.dram_tensor("xl", [lo_pairs, P, 2, m], mybir.dt.float8e4, kind="ExternalInput")
        ql_d = nc.dram_tensor("ql", [lo_pairs, P, 2, npc], mybir.dt.float8e4, kind="ExternalInput")
    a_d = nc.dram_tensor("a32", [P, npc], mybir.dt.float32, kind="ExternalInput")
    b_d = nc.dram_tensor("b32", [P, npc], mybir.dt.float32, kind="ExternalInput")
    c_d = nc.dram_tensor("c32", [P, npc], mybir.dt.float32, kind="ExternalInput")
    rs_d = nc.dram_tensor("rs", [P, m // P], mybir.dt.float32, kind="ExternalInput")
    rl_d = nc.dram_tensor("rl", [P, m // P], mybir.dt.float32, kind="ExternalInput")
    out_d = nc.dram_tensor("out", [m, npc], mybir.dt.float32, kind="ExternalOutput")

    with tile.TileContext(nc) as tc:
        with (
            tc.tile_pool(name="const", bufs=1) as const_pool,
            tc.tile_pool(name="w", bufs=1) as w_pool,
            tc.tile_pool(name="x", bufs=2) as x_pool,
            tc.tile_pool(name="o", bufs=2) as o_pool,
            tc.tile_pool(name="ps", bufs=2, space="PSUM") as ps_pool,
            tc.tile_pool(name="wps", bufs=1, space="PSUM") as warm_ps_pool,
        ):
            a32t = const_pool.tile([P, npc], mybir.dt.float32, tag="a32t")
            b32t = const_pool.tile([P, npc], mybir.dt.float32, tag="b32t")
            c32t = const_pool.tile([P, npc], mybir.dt.float32, tag="c32t")
            rs_t = const_pool.tile([P, m // P], mybir.dt.float32, tag="rs_t")
            rl_t = const_pool.tile([P, m // P], mybir.dt.float32, tag="rl_t")
            # PE warmup: flip the HAM clock gate to 8/8 while DMAs stream in.
            if warmup:
                wsrc = const_pool.tile([P, 256], mybir.dt.bfloat16, tag="wsrc")
                nc.vector.memset(wsrc[:], 0.0)
                wp = warm_ps_pool.tile([P, 256], mybir.dt.float32, tag="wp")
                for _ in range(warmup):
                    nc.tensor.matmul(wp[:], wsrc[:, :P], wsrc[:], start=True, stop=True)

            # Resident weights: 16 hi pairs + lo_pairs residual pairs, fp8.
            w_tiles = [
                w_pool.tile([P, 2, npc], mybir.dt.float8e4, name=f"W{pr}", tag=f"W{pr}")
                for pr in range(npass)
            ]
            # group-0 activations land interleaved with the weights so the
            # first passes can start while the rest stream in
            xg0 = x_pool.tile([P, npass, 2, mg], mybir.dt.float8e4, tag="xg")
            for pr in range(NPAIR):
                nc.sync.dma_start(w_tiles[pr][:], qt_d[pr])
                nc.sync.dma_start(xg0[:, pr], xt_d[pr, :, :, 0:mg])
            for j in range(lo_pairs):
                nc.sync.dma_start(w_tiles[NPAIR + j][:], ql_d[j])
                nc.sync.dma_start(xg0[:, NPAIR + j], xl_d[j, :, :, 0:mg])

            # coefficient rows are first needed at the first eviction: keep
            # their 2.1MB out of the pre-mb0 DMA critical path
            nc.sync.dma_start(a32t[:], a_d[:])
            nc.sync.dma_start(b32t[:], b_d[:])
            nc.sync.dma_start(c32t[:], c_d[:])
            nc.sync.dma_start(rs_t[:], rs_d[:])
            nc.sync.dma_start(rl_t[:], rl_d[:])

            # Main matmul loop: m-groups of `mg` tokens, 128-token m-blocks.
            def evict(ps, mbi, n0, nw, ot=None):
                """Fused dequant of psum cols [n0, n0+nw):
                   out = psum*(s/32) + (s*qbar)*rl[m] + (-s*z)*r[m]"""
                if ot is None:
                    ot = o_pool.tile([P, npc], mybir.dt.float32, name="ot", tag="ot")
                nc.scalar.copy(ot[:, n0:n0 + nw], ps[:, n0:n0 + nw])
                nc.vector.tensor_tensor(
                    ot[:, n0:n0 + nw], ot[:, n0:n0 + nw], a32t[:, n0:n0 + nw],
                    op=mybir.AluOpType.mult,
                )
                nc.vector.scalar_tensor_tensor(
                    ot[:, n0:n0 + nw], b32t[:, n0:n0 + nw],
                    rl_t[:, mbi:mbi + 1], ot[:, n0:n0 + nw],
                    op0=mybir.AluOpType.mult, op1=mybir.AluOpType.add,
                )
                nc.vector.scalar_tensor_tensor(
                    ot[:, n0:n0 + nw], c32t[:, n0:n0 + nw],
                    rs_t[:, mbi:mbi + 1], ot[:, n0:n0 + nw],
                    op0=mybir.AluOpType.mult, op1=mybir.AluOpType.add,
                )
                nc.sync.dma_start(
                    out_d[mbi * P:(mbi + 1) * P, n0:n0 + nw], ot[:, n0:n0 + nw]
                )
                return ot

            xgs = {0: xg0}
            nblocks = ngroups * mbs

            def xg_lhsT(mbi, i):
                g, mb = divmod(mbi, mbs)
                return xgs[g][:, i, :, mb * P:(mb + 1) * P]

            # blocks 0+1 in lockstep: rides the initial DMA fill compute-bound
            # (pass p of both blocks runs as soon as pair p lands)
            ps0 = ps_pool.tile([P, npc], mybir.dt.float32, name="ps0", tag="ps")
            ps1 = ps_pool.tile([P, npc], mybir.dt.float32, name="ps1", tag="ps")
            for i in range(npass):
                for ps, mbi in ((ps0, 0), (ps1, 1)):
                    for (n0, nw) in chunks:
                        nc.tensor.matmul(
                            ps[:, n0:n0 + nw], xg_lhsT(mbi, i),
                            w_tiles[i][:, :, n0:n0 + nw],
                            start=(i == 0), stop=(i == npass - 1),
                            perf_mode=DR,
                        )
            evict(ps0, 0, 0, npc)
            evict(ps1, 1, 0, npc)

            for mbi in range(2, nblocks):
                g, mb = divmod(mbi, mbs)
                if mb == 0:
                    xg = x_pool.tile(
                        [P, npass, 2, mg], mybir.dt.float8e4, name="xg", tag="xg"
                    )
                    xgs[g] = xg
                    for pr in range(NPAIR):
                        nc.sync.dma_start(
                            xg[:, pr], xt_d[pr, :, :, g * mg:(g + 1) * mg]
                        )
                    for j in range(lo_pairs):
                        nc.sync.dma_start(
                            xg[:, NPAIR + j], xl_d[j, :, :, g * mg:(g + 1) * mg]
                        )
                ps = ps_pool.tile([P, npc], mybir.dt.float32, name="ps", tag="ps")
                if mbi < nblocks - 1:
                    for i in range(npass):
                        lhsT = xg_lhsT(mbi, i)
                        for (n0, nw) in chunks:
                            nc.tensor.matmul(
                                ps[:, n0:n0 + nw], lhsT,
                                w_tiles[i][:, :, n0:n0 + nw],
                                start=(i == 0), stop=(i == npass - 1),
                                perf_mode=DR,
                            )
                    evict(ps, mbi, 0, npc)
                else:
                    # last block: chunk-major so each chunk's eviction + out-DMA
                    # overlaps the remaining chunks' matmuls (shortens the tail)
                    ot = o_pool.tile([P, npc], mybir.dt.float32, name="ot", tag="ot")
                    for (n0, nw) in chunks:
                        for i in range(npass):
                            nc.tensor.matmul(
                                ps[:, n0:n0 + nw], xg_lhsT(mbi, i),
                                w_tiles[i][:, :, n0:n0 + nw],
                                start=(i == 0), stop=(i == npass - 1),
                                perf_mode=DR,
                            )
                        evict(ps, mbi, n0, nw, ot=ot)

    nc.compile()
    return nc


def _to_pairs(arr_km, npair):
    """[M-ish rows, K' cols] -> [npair, 128, 2, rows] with
    k' = 256*pair + 128*slot + partition_row."""
    rows = arr_km.shape[0]
    t = np.ascontiguousarray(
        arr_km.T.reshape(npair, 2, P, rows).transpose(0, 2, 1, 3)
    )
    return t


def prep_inputs(inp, quant_weight, scales, zeros, ncores=NCORES, npc=NPC,
                lo_pairs=LO_PAIRS):
    """Host-side sharding/layout: returns in_maps list for run_bass_kernel_spmd."""
    m = inp.shape[0] * inp.shape[1]
    k = inp.shape[2]
    kp = k // 2
    f8 = ml_dtypes.float8_e4m3

    x = np.asarray(inp, dtype=np.float32).reshape(m, k)
    # k' permutation: k' in [0, 2048) -> even k (low nibble), [2048, 4096) -> odd
    xp = np.concatenate([x[:, 0::2], x[:, 1::2]], axis=1)  # [m, k']
    x8 = (xp * np.float32(C_HI)).astype(f8)
    xlo = xp - x8.astype(np.float32) / np.float32(C_HI)
    xt = _to_pairs(x8, NPAIR)  # [16, P, 2, m] fp8

    klo = 2 * P * lo_pairs
    if lo_pairs:
        xl8 = (xlo[:, :klo] * np.float32(C_LO)).astype(f8)
        xlt = _to_pairs(xl8, lo_pairs)  # [lo_pairs, P, 2, m] fp8
        xlo[:, :klo] -= xl8.astype(np.float32) / np.float32(C_LO)

    r = x.sum(axis=1, dtype=np.float64).astype(np.float32)    # [m] exact rowsums
    rl = xlo.sum(axis=1, dtype=np.float64).astype(np.float32)  # residual rowsums
    rs_host = np.ascontiguousarray(r.reshape(m // P, P).T)    # [P, m//P]
    rl_host = np.ascontiguousarray(rl.reshape(m // P, P).T)

    n = quant_weight.shape[0]
    assert n == ncores * npc, (n, ncores, npc)
    qb = np.asarray(quant_weight).astype(np.uint8)
    low = (qb & 15).astype(np.float32)
    high = ((qb >> 4) & 15).astype(np.float32)
    qp = np.concatenate([low, high], axis=1)  # [n, k'] nibble values
    qbar = qp.mean(axis=1, dtype=np.float64).astype(np.float32)  # [n] exact

    s_all = np.asarray(scales, dtype=np.float32).reshape(-1)
    z_all = np.asarray(zeros, dtype=np.float32).reshape(-1)
    a_all = s_all / np.float32(C_HI)
    b_all = s_all * qbar
    c_all = -(s_all * z_all)

    in_maps = []
    for c in range(ncores):
        sl = slice(c * npc, (c + 1) * npc)
        qpc = qp[sl]
        qt_c = _to_pairs(qpc.astype(f8), NPAIR)  # [16, P, 2, npc] fp8, exact
        im = {
            "xt": xt, "qt": qt_c,
            "a32": np.ascontiguousarray(np.broadcast_to(a_all[sl], (P, npc))),
            "b32": np.ascontiguousarray(np.broadcast_to(b_all[sl], (P, npc))),
            "c32": np.ascontiguousarray(np.broadcast_to(c_all[sl], (P, npc))),
            "rs": rs_host, "rl": rl_host,
        }
        if lo_pairs:
            # residual pass moving operand: q/16, exact in e4m3
            im["xl"] = xlt
            im["ql"] = _to_pairs(
                (qpc[:, :klo] / np.float32(16.0)).astype(f8), lo_pairs
            )
        in_maps.append(im)
    return in_maps


_NC_CACHE = {}


def _get_nc():
    if "nc" not in _NC_CACHE:
        _NC_CACHE["nc"] = build_nc()
    return _NC_CACHE["nc"]


def kernel(inp, quant_weight, scales, zeros):
    from concourse.bass_utils import run_bass_kernel_spmd

    nc = _get_nc()
    in_maps = prep_inputs(inp, quant_weight, scales, zeros)
    res = run_bass_kernel_spmd(nc, in_maps, list(range(NCORES)))
    out = np.concatenate([res.results[c]["out"] for c in range(NCORES)], axis=1)
    return np.ascontiguousarray(out).reshape(B, S, N)


# revision 12
# speedup vs baseline: 1.9601x; 1.0589x over previous
"""4-bit column-block-quantized linear (ColBlockQuantizedLinear) on 8 Trainium2 NeuronCores.

Reference computation:
    w[n, k] = (nibble(quant_weight)[n, k] - zeros[n]) * scales[n]     n<11008, k<4096
    out[b, s, n] = sum_k inp[b, s, k] * w[n, k]                        inp: [4, 2048, 4096] f32

Strategy (column-parallel, per sharding hint), fp8 DoubleRow edition:
  - Shard out_features N=11008 = 8*1376 across 8 cores; replicate inp.
  - The nibble weights q in {0..15} are EXACT in fp8-e4m3.  Activations are
    quantized host-side to x8 = e4m3(32*x).  The PE then runs in DoubleRow
    perf mode (two 128-row k-tiles per pass, 2x bf16 throughput):
        psum[m, n] += sum_i x8[kpair, :, i, m].T @ q[kpair, :, i, n]
    over 16 k-pairs (K = 4096 = 16 pairs * 256).
  - Error control: the activation-quantization residual xlo = x - x8/32 is
    handled two ways:  (a) for the first L k-pairs an exact second DoubleRow
    pass with xlo8 = e4m3(512*xlo) against q/16 (also exact in fp8)
    accumulates into the SAME psum group;  (b) the remaining residual is
    corrected at rank 1 with exact per-token residual rowsums against the
    exact per-channel mean nibble qbar_n.
  - Eviction fuses the dequant:  out = psum*(s/32) + (s*qbar)*rl[m] + (-s*z)*r[m]
    with r = exact rowsums of x, rl = rowsums of the remaining residual.
  - A short burst of dummy matmuls warms the PE (HAM un-throttle) while the
    first activation tiles and weights stream in.
  - Host concatenates per-core outputs along N.
"""

import sys

for _p in ("/opt/trn_rl_repo", "/opt/pypackages"):
    if _p not in sys.path:
        sys.path.append(_p)

import numpy as np
import ml_dtypes

import concourse.bass as bass
import concourse.mybir as mybir
import concourse.tile as tile
from concourse import bacc

# Problem constants (hardcoded per harness contract)
B, S, K = 4, 2048, 4096
M = B * S                  # 8192 tokens
N = 11008                  # out features
NCORES = 8
NPC = N // NCORES          # per-core out features (1376)
P = 128
NPAIR = K // (2 * P)       # 16 DoubleRow k-pairs
LO_PAIRS = 0               # k-pairs that get an exact fp8 residual pass
C_HI = 32.0                # x8 = e4m3(C_HI * x)
C_LO = 512.0               # xlo8 = e4m3(C_LO * (x - x8/C_HI)); C_LO/C_HI = 16


def _nchunks(npc, step=512):
    return [(i, min(step, npc - i)) for i in range(0, npc, step)]


def build_nc(m=M, npc=NPC, lo_pairs=LO_PAIRS, mg=1024, warmup=24):
    """Build the per-core Bass program. m tokens, npc out cols, mg tokens per
    m-group (DMA granule)."""
    ngroups = m // mg
    mbs = mg // P              # m-blocks per group
    chunks = _nchunks(npc)
    npass = NPAIR + lo_pairs
    DR = mybir.MatmulPerfMode.DoubleRow

    nc = bacc.Bacc("TRN2", target_bir_lowering=False, debug=False)
    xt_d = nc.dram_tensor("xt", [NPAIR, P, 2, m], mybir.dt.float8e4, kind="ExternalInput")
    qt_d = nc.dram_tensor("qt", [NPAIR, P, 2, npc], mybir.dt.float8e4, kind="ExternalInput")
    if lo_pairs:
        xl_d = nc.dram_tensor("xl", [lo_pairs, P, 2, m], mybir.dt.float8e4, kind="ExternalInput")
        ql_d = nc.dram_tensor("ql", [lo_pairs, P, 2, npc], mybir.dt.float8e4, kind="ExternalInput")
    a_d = nc.dram_tensor("a32", [P, npc], mybir.dt.float32, kind="ExternalInput")
    b_d = nc.dram_tensor("b32", [P, npc], mybir.dt.float32, kind="ExternalInput")
    c_d = nc.dram_tensor("c32", [P, npc], mybir.dt.float32, kind="ExternalInput")
    rs_d = nc.dram_tensor("rs", [P, m // P], mybir.dt.float32, kind="ExternalInput")
    rl_d = nc.dram_tensor("rl", [P, m // P], mybir.dt.float32, kind="ExternalInput")
    out_d = nc.dram_tensor("out", [m, npc], mybir.dt.float32, kind="ExternalOutput")

    with tile.TileContext(nc) as tc:
        with (
            tc.tile_pool(name="const", bufs=1) as const_pool,
            tc.tile_pool(name="w", bufs=1) as w_pool,
            tc.tile_pool(name="x", bufs=2) as x_pool,
            tc.tile_pool(name="o", bufs=2) as o_pool,
            tc.tile_pool(name="ps", bufs=2, space="PSUM") as ps_pool,
            tc.tile_pool(name="wps", bufs=1, space="PSUM") as warm_ps_pool,
        ):
            a32t = const_pool.tile([P, npc], mybir.dt.float32, tag="a32t")
            b32t = const_pool.tile([P, npc], mybir.dt.float32, tag="b32t")
            c32t = const_pool.tile([P, npc], mybir.dt.float32, tag="c32t")
            rs_t = const_pool.tile([P, m // P], mybir.dt.float32, tag="rs_t")
            rl_t = const_pool.tile([P, m // P], mybir.dt.float32, tag="rl_t")
            # PE warmup: flip the HAM clock gate to 8/8 while DMAs stream in.
            if warmup:
                wsrc = const_pool.tile([P, 256], mybir.dt.bfloat16, tag="wsrc")
                nc.vector.memset(wsrc[:], 0.0)
                wp = warm_ps_pool.tile([P, 256], mybir.dt.float32, tag="wp")
                for _ in range(warmup):
                    nc.tensor.matmul(wp[:], wsrc[:, :P], wsrc[:], start=True, stop=True)

            # Resident weights: 16 hi pairs + lo_pairs residual pairs, fp8.
            w_tiles = [
                w_pool.tile([P, 2, npc], mybir.dt.float8e4, name=f"W{pr}", tag=f"W{pr}")
                for pr in range(npass)
            ]
            # group-0 activations land interleaved with the weights so the
            # first passes can start while the rest stream in
            xg0 = x_pool.tile([P, npass, 2, mg], mybir.dt.float8e4, tag="xg")
            for pr in range(NPAIR):
                nc.sync.dma_start(w_tiles[pr][:], qt_d[pr])
                nc.sync.dma_start(xg0[:, pr], xt_d[pr, :, :, 0:mg])
            for j in range(lo_pairs):
                nc.sync.dma_start(w_tiles[NPAIR + j][:], ql_d[j])
                nc.sync.dma_start(xg0[:, NPAIR + j], xl_d[j, :, :, 0:mg])

            # coefficient rows are first needed at the first eviction: keep
            # their 2.1MB out of the pre-mb0 DMA critical path
            nc.sync.dma_start(a32t[:], a_d[:])
            nc.sync.dma_start(b32t[:], b_d[:])
            nc.sync.dma_start(c32t[:], c_d[:])
            nc.sync.dma_start(rs_t[:], rs_d[:])
            nc.sync.dma_start(rl_t[:], rl_d[:])

            # Main matmul loop: m-groups of `mg` tokens, 128-token m-blocks.
            def evict(ps_ap, mbi, n0, nw, ot=None):
                """Fused dequant of psum cols [n0, n0+nw) (ps_ap pre-sliced):
                   out = psum*(s/32) + (s*qbar)*rl[m] + (-s*z)*r[m]"""
                if ot is None:
                    ot = o_pool.tile([P, npc], mybir.dt.float32, name="ot", tag="ot")
                nc.scalar.copy(ot[:, n0:n0 + nw], ps_ap)
                nc.vector.tensor_tensor(
                    ot[:, n0:n0 + nw], ot[:, n0:n0 + nw], a32t[:, n0:n0 + nw],
                    op=mybir.AluOpType.mult,
                )
                nc.vector.scalar_tensor_tensor(
                    ot[:, n0:n0 + nw], b32t[:, n0:n0 + nw],
                    rl_t[:, mbi:mbi + 1], ot[:, n0:n0 + nw],
                    op0=mybir.AluOpType.mult, op1=mybir.AluOpType.add,
                )
                nc.vector.scalar_tensor_tensor(
                    ot[:, n0:n0 + nw], c32t[:, n0:n0 + nw],
                    rs_t[:, mbi:mbi + 1], ot[:, n0:n0 + nw],
                    op0=mybir.AluOpType.mult, op1=mybir.AluOpType.add,
                )
                nc.sync.dma_start(
                    out_d[mbi * P:(mbi + 1) * P, n0:n0 + nw], ot[:, n0:n0 + nw]
                )
                return ot

            xgs = {0: xg0}
            nblocks = ngroups * mbs

            def xg_lhsT(mbi, i):
                g, mb = divmod(mbi, mbs)
                return xgs[g][:, i, :, mb * P:(mb + 1) * P]

            # blocks 0+1 in lockstep: rides the initial DMA fill compute-bound
            # (pass p of both blocks runs as soon as pair p lands)
            ps0 = ps_pool.tile([P, npc], mybir.dt.float32, name="ps0", tag="ps")
            ps1 = ps_pool.tile([P, npc], mybir.dt.float32, name="ps1", tag="ps")
            for i in range(npass):
                for ps, mbi in ((ps0, 0), (ps1, 1)):
                    for (n0, nw) in chunks:
                        nc.tensor.matmul(
                            ps[:, n0:n0 + nw], xg_lhsT(mbi, i),
                            w_tiles[i][:, :, n0:n0 + nw],
                            start=(i == 0), stop=(i == npass - 1),
                            perf_mode=DR,
                        )
            evict(ps0[:], 0, 0, npc)
            evict(ps1[:], 1, 0, npc)

            for mbi in range(2, nblocks):
                g, mb = divmod(mbi, mbs)
                if mb == 0:
                    xg = x_pool.tile(
                        [P, npass, 2, mg], mybir.dt.float8e4, name="xg", tag="xg"
                    )
                    xgs[g] = xg
                    for pr in range(NPAIR):
                        nc.sync.dma_start(
                            xg[:, pr], xt_d[pr, :, :, g * mg:(g + 1) * mg]
                        )
                    for j in range(lo_pairs):
                        nc.sync.dma_start(
                            xg[:, NPAIR + j], xl_d[j, :, :, g * mg:(g + 1) * mg]
                        )
                if mbi < nblocks - 1:
                    ps = ps_pool.tile([P, npc], mybir.dt.float32, name="ps", tag="ps")
                    for i in range(npass):
                        lhsT = xg_lhsT(mbi, i)
                        for (n0, nw) in chunks:
                            nc.tensor.matmul(
                                ps[:, n0:n0 + nw], lhsT,
                                w_tiles[i][:, :, n0:n0 + nw],
                                start=(i == 0), stop=(i == npass - 1),
                                perf_mode=DR,
                            )
                    evict(ps[:], mbi, 0, npc)
                else:
                    # last block: chunk-major with a per-chunk psum tile, so
                    # each chunk's eviction + out-DMA overlaps the remaining
                    # chunks' matmuls with no whole-tile WAR hazard between
                    # chunks (shortens the kernel tail)
                    ot = o_pool.tile([P, npc], mybir.dt.float32, name="ot", tag="ot")
                    for (n0, nw) in chunks:
                        psc = ps_pool.tile(
                            [P, nw], mybir.dt.float32, name=f"psL{n0}", tag="ps"
                        )
                        for i in range(npass):
                            nc.tensor.matmul(
                                psc[:], xg_lhsT(mbi, i),
                                w_tiles[i][:, :, n0:n0 + nw],
                                start=(i == 0), stop=(i == npass - 1),
                                perf_mode=DR,
                            )
                        evict(psc[:], mbi, n0, nw, ot=ot)

    nc.compile()
    return nc


def _to_pairs(arr_km, npair):
    """[M-ish rows, K' cols] -> [npair, 128, 2, rows] with
    k' = 256*pair + 128*slot + partition_row."""
    rows = arr_km.shape[0]
    t = np.ascontiguousarray(
        arr_km.T.reshape(npair, 2, P, rows).transpose(0, 2, 1, 3)
    )
    return t


def prep_inputs(inp, quant_weight, scales, zeros, ncores=NCORES, npc=NPC,
                lo_pairs=LO_PAIRS):
    """Host-side sharding/layout: returns in_maps list for run_bass_kernel_spmd."""
    m = inp.shape[0] * inp.shape[1]
    k = inp.shape[2]
    kp = k // 2
    f8 = ml_dtypes.float8_e4m3

    x = np.asarray(inp, dtype=np.float32).reshape(m, k)
    # k' permutation: k' in [0, 2048) -> even k (low nibble), [2048, 4096) -> odd
    xp = np.concatenate([x[:, 0::2], x[:, 1::2]], axis=1)  # [m, k']
    x8 = (xp * np.float32(C_HI)).astype(f8)
    xlo = xp - x8.astype(np.float32) / np.float32(C_HI)
    xt = _to_pairs(x8, NPAIR)  # [16, P, 2, m] fp8

    klo = 2 * P * lo_pairs
    if lo_pairs:
        xl8 = (xlo[:, :klo] * np.float32(C_LO)).astype(f8)
        xlt = _to_pairs(xl8, lo_pairs)  # [lo_pairs, P, 2, m] fp8
        xlo[:, :klo] -= xl8.astype(np.float32) / np.float32(C_LO)

    r = x.sum(axis=1, dtype=np.float64).astype(np.float32)    # [m] exact rowsums
    rl = xlo.sum(axis=1, dtype=np.float64).astype(np.float32)  # residual rowsums
    rs_host = np.ascontiguousarray(r.reshape(m // P, P).T)    # [P, m//P]
    rl_host = np.ascontiguousarray(rl.reshape(m // P, P).T)

    n = quant_weight.shape[0]
    assert n == ncores * npc, (n, ncores, npc)
    qb = np.asarray(quant_weight).astype(np.uint8)
    low = (qb & 15).astype(np.float32)
    high = ((qb >> 4) & 15).astype(np.float32)
    qp = np.concatenate([low, high], axis=1)  # [n, k'] nibble values
    qbar = qp.mean(axis=1, dtype=np.float64).astype(np.float32)  # [n] exact

    s_all = np.asarray(scales, dtype=np.float32).reshape(-1)
    z_all = np.asarray(zeros, dtype=np.float32).reshape(-1)
    a_all = s_all / np.float32(C_HI)
    b_all = s_all * qbar
    c_all = -(s_all * z_all)

    in_maps = []
    for c in range(ncores):
        sl = slice(c * npc, (c + 1) * npc)
        qpc = qp[sl]
        qt_c = _to_pairs(qpc.astype(f8), NPAIR)  # [16, P, 2, npc] fp8, exact
        im = {
            "xt": xt, "qt": qt_c,
            "a32": np.ascontiguousarray(np.broadcast_to(a_all[sl], (P, npc))),
            "b32": np.ascontiguousarray(np.broadcast_to(b_all[sl], (P, npc))),
            "c32": np.ascontiguousarray(np.broadcast_to(c_all[sl], (P, npc))),
            "rs": rs_host, "rl": rl_host,
        }
        if lo_pairs:
            # residual pass moving operand: q/16, exact in e4m3
            im["xl"] = xlt
            im["ql"] = _to_pairs(
                (qpc[:, :klo] / np.float32(16.0)).astype(f8), lo_pairs
            )
        in_maps.append(im)
    return in_maps


_NC_CACHE = {}


def _get_nc():
    if "nc" not in _NC_CACHE:
        _NC_CACHE["nc"] = build_nc()
    return _NC_CACHE["nc"]


def kernel(inp, quant_weight, scales, zeros):
    from concourse.bass_utils import run_bass_kernel_spmd

    nc = _get_nc()
    in_maps = prep_inputs(inp, quant_weight, scales, zeros)
    res = run_bass_kernel_spmd(nc, in_maps, list(range(NCORES)))
    out = np.concatenate([res.results[c]["out"] for c in range(NCORES)], axis=1)
    return np.ascontiguousarray(out).reshape(B, S, N)


# revision 18
# speedup vs baseline: 1.9714x; 1.0058x over previous
"""4-bit column-block-quantized linear (ColBlockQuantizedLinear) on 8 Trainium2 NeuronCores.

Reference computation:
    w[n, k] = (nibble(quant_weight)[n, k] - zeros[n]) * scales[n]     n<11008, k<4096
    out[b, s, n] = sum_k inp[b, s, k] * w[n, k]                        inp: [4, 2048, 4096] f32

Strategy (column-parallel, per sharding hint), fp8 DoubleRow edition:
  - Shard out_features N=11008 = 8*1376 across 8 cores; replicate inp.
  - The nibble weights q in {0..15} are EXACT in fp8-e4m3.  Activations are
    quantized host-side to x8 = e4m3(32*x).  The PE then runs in DoubleRow
    perf mode (two 128-row k-tiles per pass, 2x bf16 throughput):
        psum[m, n] += sum_i x8[kpair, :, i, m].T @ q[kpair, :, i, n]
    over 16 k-pairs (K = 4096 = 16 pairs * 256).
  - Error control: the activation-quantization residual xlo = x - x8/32 is
    handled two ways:  (a) for the first L k-pairs an exact second DoubleRow
    pass with xlo8 = e4m3(512*xlo) against q/16 (also exact in fp8)
    accumulates into the SAME psum group;  (b) the remaining residual is
    corrected at rank 1 with exact per-token residual rowsums against the
    exact per-channel mean nibble qbar_n.
  - Eviction fuses the dequant:  out = psum*(s/32) + (s*qbar)*rl[m] + (-s*z)*r[m]
    with r = exact rowsums of x, rl = rowsums of the remaining residual.
  - A short burst of dummy matmuls warms the PE (HAM un-throttle) while the
    first activation tiles and weights stream in.
  - Host concatenates per-core outputs along N.
"""

import sys

for _p in ("/opt/trn_rl_repo", "/opt/pypackages"):
    if _p not in sys.path:
        sys.path.append(_p)

import numpy as np
import ml_dtypes

import concourse.bass as bass
import concourse.mybir as mybir
import concourse.tile as tile
from concourse import bacc

# Problem constants (hardcoded per harness contract)
B, S, K = 4, 2048, 4096
M = B * S                  # 8192 tokens
N = 11008                  # out features
NCORES = 8
NPC = N // NCORES          # per-core out features (1376)
P = 128
NPAIR = K // (2 * P)       # 16 DoubleRow k-pairs
LO_PAIRS = 0               # k-pairs that get an exact fp8 residual pass
C_HI = 32.0                # x8 = e4m3(C_HI * x)
C_LO = 512.0               # xlo8 = e4m3(C_LO * (x - x8/C_HI)); C_LO/C_HI = 16


def _nchunks(npc, step=512):
    return [(i, min(step, npc - i)) for i in range(0, npc, step)]


def build_nc(m=M, npc=NPC, lo_pairs=LO_PAIRS, mg=1024, warmup=24):
    """Build the per-core Bass program. m tokens, npc out cols, mg tokens per
    m-group (DMA granule)."""
    ngroups = m // mg
    mbs = mg // P              # m-blocks per group
    chunks = _nchunks(npc)
    npass = NPAIR + lo_pairs
    DR = mybir.MatmulPerfMode.DoubleRow

    nc = bacc.Bacc("TRN2", target_bir_lowering=False, debug=False)
    xt_d = nc.dram_tensor("xt", [NPAIR, P, 2, m], mybir.dt.float8e4, kind="ExternalInput")
    # packed nibble weights: tile t rows kk in [128t,128t+128), byte = lo | hi<<4
    qp_d = nc.dram_tensor("qp", [NPAIR, P, npc], mybir.dt.uint8, kind="ExternalInput")
    if lo_pairs:
        xl_d = nc.dram_tensor("xl", [lo_pairs, P, 2, m], mybir.dt.float8e4, kind="ExternalInput")
        ql_d = nc.dram_tensor("ql", [lo_pairs, P, 2, npc], mybir.dt.float8e4, kind="ExternalInput")
    a_d = nc.dram_tensor("a32", [P, npc], mybir.dt.float32, kind="ExternalInput")
    b_d = nc.dram_tensor("b32", [P, npc], mybir.dt.float32, kind="ExternalInput")
    c_d = nc.dram_tensor("c32", [P, npc], mybir.dt.float32, kind="ExternalInput")
    rs_d = nc.dram_tensor("rs", [P, m // P], mybir.dt.float32, kind="ExternalInput")
    rl_d = nc.dram_tensor("rl", [P, m // P], mybir.dt.float32, kind="ExternalInput")
    out_d = nc.dram_tensor("out", [m, npc], mybir.dt.float32, kind="ExternalOutput")

    with tile.TileContext(nc) as tc:
        with (
            tc.tile_pool(name="const", bufs=1) as const_pool,
            tc.tile_pool(name="w", bufs=1) as w_pool,
            tc.tile_pool(name="x", bufs=2) as x_pool,
            tc.tile_pool(name="o", bufs=2) as o_pool,
            tc.tile_pool(name="d", bufs=3) as d_pool,
            tc.tile_pool(name="ps", bufs=2, space="PSUM") as ps_pool,
            tc.tile_pool(name="wps", bufs=1, space="PSUM") as warm_ps_pool,
        ):
            a32t = const_pool.tile([P, npc], mybir.dt.float32, tag="a32t")
            b32t = const_pool.tile([P, npc], mybir.dt.float32, tag="b32t")
            c32t = const_pool.tile([P, npc], mybir.dt.float32, tag="c32t")
            rs_t = const_pool.tile([P, m // P], mybir.dt.float32, tag="rs_t")
            rl_t = const_pool.tile([P, m // P], mybir.dt.float32, tag="rl_t")
            # PE warmup: flip the HAM clock gate to 8/8 while DMAs stream in.
            if warmup:
                wsrc = const_pool.tile([P, 256], mybir.dt.bfloat16, tag="wsrc")
                nc.vector.memset(wsrc[:], 0.0)
                wp = warm_ps_pool.tile([P, 256], mybir.dt.float32, tag="wp")
                for _ in range(warmup):
                    nc.tensor.matmul(wp[:], wsrc[:, :P], wsrc[:], start=True, stop=True)

            # Resident weights: 16 hi pairs + lo_pairs residual pairs, fp8.
            w_tiles = [
                w_pool.tile([P, 2, npc], mybir.dt.float8e4, name=f"W{pr}", tag=f"W{pr}")
                for pr in range(npass)
            ]
            # group-0 activations land interleaved with the weights so the
            # first passes can start while the rest stream in
            xg0 = x_pool.tile([P, npass, 2, mg], mybir.dt.float8e4, tag="xg")
            for pr in range(NPAIR):
                nc.sync.dma_start(w_tiles[pr][:], qt_d[pr])
                nc.sync.dma_start(xg0[:, pr], xt_d[pr, :, :, 0:mg])
            for j in range(lo_pairs):
                nc.sync.dma_start(w_tiles[NPAIR + j][:], ql_d[j])
                nc.sync.dma_start(xg0[:, NPAIR + j], xl_d[j, :, :, 0:mg])

            # coefficient rows are first needed at the first eviction: keep
            # their 2.1MB out of the pre-mb0 DMA critical path
            nc.sync.dma_start(a32t[:], a_d[:])
            nc.sync.dma_start(b32t[:], b_d[:])
            nc.sync.dma_start(c32t[:], c_d[:])
            nc.sync.dma_start(rs_t[:], rs_d[:])
            nc.sync.dma_start(rl_t[:], rl_d[:])

            # Main matmul loop: m-groups of `mg` tokens, 128-token m-blocks.
            # Per-block rank-2 correction D = (s*qbar)*rl[m] + (-s*z)*r[m] is
            # precomputed ahead of the block's stop-matmul (no psum dependency
            # -> overlaps matmuls in the in-order DVE queue), so the
            # post-matmul eviction chain is just two DVE ops:
            #   ot = psum*(s/32);  ot += D
            # (TensorScalarPtr is DVE-only on core V3 — Pool engine rejects it.)
            def make_d(mbi):
                dt_ = d_pool.tile([P, npc], mybir.dt.float32, name="D", tag="D")
                nc.vector.tensor_scalar(
                    dt_[:], c32t[:], rs_t[:, mbi:mbi + 1], None,
                    op0=mybir.AluOpType.mult,
                )
                nc.vector.scalar_tensor_tensor(
                    dt_[:], b32t[:], rl_t[:, mbi:mbi + 1], dt_[:],
                    op0=mybir.AluOpType.mult, op1=mybir.AluOpType.add,
                )
                return dt_

            def evict(ps_ap, dt_, mbi, n0, nw, ot=None):
                if ot is None:
                    ot = o_pool.tile([P, npc], mybir.dt.float32, name="ot", tag="ot")
                nc.vector.tensor_tensor(
                    ot[:, n0:n0 + nw], ps_ap, a32t[:, n0:n0 + nw],
                    op=mybir.AluOpType.mult,
                )
                nc.vector.tensor_tensor(
                    ot[:, n0:n0 + nw], ot[:, n0:n0 + nw], dt_[:, n0:n0 + nw],
                    op=mybir.AluOpType.add,
                )
                nc.sync.dma_start(
                    out_d[mbi * P:(mbi + 1) * P, n0:n0 + nw], ot[:, n0:n0 + nw]
                )
                return ot

            xgs = {0: xg0}
            nblocks = ngroups * mbs

            def xg_lhsT(mbi, i):
                g, mb = divmod(mbi, mbs)
                return xgs[g][:, i, :, mb * P:(mb + 1) * P]

            # blocks 0+1 in lockstep: rides the initial DMA fill compute-bound
            # (pass p of both blocks runs as soon as pair p lands)
            ps0 = ps_pool.tile([P, npc], mybir.dt.float32, name="ps0", tag="ps")
            ps1 = ps_pool.tile([P, npc], mybir.dt.float32, name="ps1", tag="ps")
            d0 = make_d(0)
            d1 = make_d(1)
            for i in range(npass):
                for ps, mbi in ((ps0, 0), (ps1, 1)):
                    for (n0, nw) in chunks:
                        nc.tensor.matmul(
                            ps[:, n0:n0 + nw], xg_lhsT(mbi, i),
                            w_tiles[i][:, :, n0:n0 + nw],
                            start=(i == 0), stop=(i == npass - 1),
                            perf_mode=DR,
                        )
            evict(ps0[:], d0, 0, 0, npc)
            evict(ps1[:], d1, 1, 0, npc)

            for mbi in range(2, nblocks):
                g, mb = divmod(mbi, mbs)
                if mb == 0:
                    xg = x_pool.tile(
                        [P, npass, 2, mg], mybir.dt.float8e4, name="xg", tag="xg"
                    )
                    xgs[g] = xg
                    for pr in range(NPAIR):
                        nc.sync.dma_start(
                            xg[:, pr], xt_d[pr, :, :, g * mg:(g + 1) * mg]
                        )
                    for j in range(lo_pairs):
                        nc.sync.dma_start(
                            xg[:, NPAIR + j], xl_d[j, :, :, g * mg:(g + 1) * mg]
                        )
                dt_ = make_d(mbi)
                if mbi < nblocks - 1:
                    ps = ps_pool.tile([P, npc], mybir.dt.float32, name="ps", tag="ps")
                    for i in range(npass):
                        lhsT = xg_lhsT(mbi, i)
                        for (n0, nw) in chunks:
                            nc.tensor.matmul(
                                ps[:, n0:n0 + nw], lhsT,
                                w_tiles[i][:, :, n0:n0 + nw],
                                start=(i == 0), stop=(i == npass - 1),
                                perf_mode=DR,
                            )
                    evict(ps[:], dt_, mbi, 0, npc)
                else:
                    # last block: chunk-major with a per-chunk psum tile, so
                    # each chunk's eviction + out-DMA overlaps the remaining
                    # chunks' matmuls with no whole-tile WAR hazard between
                    # chunks (shortens the kernel tail)
                    ot = o_pool.tile([P, npc], mybir.dt.float32, name="ot", tag="ot")
                    for (n0, nw) in chunks:
                        psc = ps_pool.tile(
                            [P, nw], mybir.dt.float32, name=f"psL{n0}", tag="ps"
                        )
                        for i in range(npass):
                            nc.tensor.matmul(
                                psc[:], xg_lhsT(mbi, i),
                                w_tiles[i][:, :, n0:n0 + nw],
                                start=(i == 0), stop=(i == npass - 1),
                                perf_mode=DR,
                            )
                        evict(psc[:], dt_, mbi, n0, nw, ot=ot)

    nc.compile()
    return nc


def _to_pairs(arr_km, npair):
    """[M-ish rows, K' cols] -> [npair, 128, 2, rows] with
    k' = 256*pair + 128*slot + partition_row."""
    rows = arr_km.shape[0]
    t = np.ascontiguousarray(
        arr_km.T.reshape(npair, 2, P, rows).transpose(0, 2, 1, 3)
    )
    return t


def prep_inputs(inp, quant_weight, scales, zeros, ncores=NCORES, npc=NPC,
                lo_pairs=LO_PAIRS):
    """Host-side sharding/layout: returns in_maps list for run_bass_kernel_spmd."""
    m = inp.shape[0] * inp.shape[1]
    k = inp.shape[2]
    kp = k // 2
    f8 = ml_dtypes.float8_e4m3

    x = np.asarray(inp, dtype=np.float32).reshape(m, k)
    # k' permutation: k' in [0, 2048) -> even k (low nibble), [2048, 4096) -> odd
    xp = np.concatenate([x[:, 0::2], x[:, 1::2]], axis=1)  # [m, k']
    x8 = (xp * np.float32(C_HI)).astype(f8)
    xlo = xp - x8.astype(np.float32) / np.float32(C_HI)
    xt = _to_pairs(x8, NPAIR)  # [16, P, 2, m] fp8

    klo = 2 * P * lo_pairs
    if lo_pairs:
        xl8 = (xlo[:, :klo] * np.float32(C_LO)).astype(f8)
        xlt = _to_pairs(xl8, lo_pairs)  # [lo_pairs, P, 2, m] fp8
        xlo[:, :klo] -= xl8.astype(np.float32) / np.float32(C_LO)

    r = x.sum(axis=1, dtype=np.float64).astype(np.float32)    # [m] exact rowsums
    rl = xlo.sum(axis=1, dtype=np.float64).astype(np.float32)  # residual rowsums
    rs_host = np.ascontiguousarray(r.reshape(m // P, P).T)    # [P, m//P]
    rl_host = np.ascontiguousarray(rl.reshape(m // P, P).T)

    n = quant_weight.shape[0]
    assert n == ncores * npc, (n, ncores, npc)
    qb = np.asarray(quant_weight).astype(np.uint8)
    low = (qb & 15).astype(np.float32)
    high = ((qb >> 4) & 15).astype(np.float32)
    qp = np.concatenate([low, high], axis=1)  # [n, k'] nibble values
    qbar = qp.mean(axis=1, dtype=np.float64).astype(np.float32)  # [n] exact

    s_all = np.asarray(scales, dtype=np.float32).reshape(-1)
    z_all = np.asarray(zeros, dtype=np.float32).reshape(-1)
    a_all = s_all / np.float32(C_HI)
    b_all = s_all * qbar
    c_all = -(s_all * z_all)

    in_maps = []
    for c in range(ncores):
        sl = slice(c * npc, (c + 1) * npc)
        qpc = qp[sl]
        qt_c = _to_pairs(qpc.astype(f8), NPAIR)  # [16, P, 2, npc] fp8, exact
        im = {
            "xt": xt, "qt": qt_c,
            "a32": np.ascontiguousarray(np.broadcast_to(a_all[sl], (P, npc))),
            "b32": np.ascontiguousarray(np.broadcast_to(b_all[sl], (P, npc))),
            "c32": np.ascontiguousarray(np.broadcast_to(c_all[sl], (P, npc))),
            "rs": rs_host, "rl": rl_host,
        }
        if lo_pairs:
            # residual pass moving operand: q/16, exact in e4m3
            im["xl"] = xlt
            im["ql"] = _to_pairs(
                (qpc[:, :klo] / np.float32(16.0)).astype(f8), lo_pairs
            )
        in_maps.append(im)
    return in_maps


_NC_CACHE = {}


def _get_nc():
    if "nc" not in _NC_CACHE:
        _NC_CACHE["nc"] = build_nc()
    return _NC_CACHE["nc"]


def kernel(inp, quant_weight, scales, zeros):
    from concourse.bass_utils import run_bass_kernel_spmd

    nc = _get_nc()
    in_maps = prep_inputs(inp, quant_weight, scales, zeros)
    res = run_bass_kernel_spmd(nc, in_maps, list(range(NCORES)))
    out = np.concatenate([res.results[c]["out"] for c in range(NCORES)], axis=1)
    return np.ascontiguousarray(out).reshape(B, S, N)
